# revision 79
# baseline (speedup 1.0000x reference)
"""DiffusionNetBlock on 8 Trainium2 NeuronCores.

Strategy (data-parallel over batch x row-halves, 8 cores = 4 batches x 2):
  core c = 2*b + h owns batch b and half of its mesh vertices.

Host-side prep (sharding/layout only, no model math beyond input folding):
  - fold vertex_areas into x_in, precompute the spectral heat scale
    exp(-evals x times) (tiny [K,P] per batch), transpose weights.
  - the sparse gradient (COO, E=160k edges/batch) is laid out for the
    device: rows of each batch are degree-sorted into 128-row blocks,
    blocks dealt to the two cores, and each block padded to a fixed
    per-slot degree D (equalized across cores so one NEFF serves all 8).
    Edges become dense fp8 streams evg = evecs[col] tiled [128 edges, K];
    the segment-sum over rows is a matmul with a small fp8 selector
    (gradient vals scattered in a 0/1-style pattern per degree bucket),
    fully on the PE with f32 PSUM accumulation.

Device kernel (Bass/Tile, same program on all 8 cores), two phases kept
PE-dense so the HAM clock gate stays at 2.4 GHz:
  phase1: C1 selector matmuls accumulate agX/agY = evecs^T-weighted
     segment sums, [K, rows] f16 resident in SBUF; phase A octets
     (x_spec += evecs_chunk^T @ (a*x)_chunk) are interleaved between
     row groups; evsT/xinT prefetch into resident SBUF tiles.
  fold: s2 = exp(-lam t) * x_spec; s2T via PE transpose; fold
     bfx = s2 @ B_re^T, bfy = s2 @ B_im^T, wf = s2 @ W1b^T so that
     x_diffuse is never materialized (phase B deleted).
  phase2 per 512-row group: gx^T = s2^T agX, Bgx^T = bfx^T agX (ditto Y),
     xg = tanh(gx.Bgx + gy.Bgy), 3-layer MLP on [x_in; evsT-fold; xg],
     + residual, f16 store.
"""

import math
import os
import sys

import numpy as np

sys.path.insert(0, "/opt/trn_rl_repo")

import ml_dtypes  # noqa: E402

from concourse import bass, mybir  # noqa: E402
from concourse import bass_utils  # noqa: E402
from concourse.tile import TileContext  # noqa: E402
from concourse.vector_clock import ScopedClock, VectorClock  # noqa: E402

B, N, P, K, E = 4, 20000, 128, 128, 160000
NCORES = 8
NBLK = 79                    # 128-row blocks per core
ROWS = NBLK * 128            # 10112 row slots per core
TOTBLK = 2 * NBLK            # 158 blocks per batch (20224 >= 20000 row slots)
GRP = 4                      # blocks per 512-wide processing group
NCHUNK = (N + 127) // 128    # 157 n-chunks for phase A (20096 padded)
NPAD = NCHUNK * 128
NGRP = NBLK // GRP + (1 if NBLK % GRP else 0)
NCHALF = NCHUNK              # phase-A chunks per core (no pair-split:
                             # SBUF collectives are broken in this toolchain)

f32 = mybir.dt.float32
f32r = mybir.dt.float32r
f16 = mybir.dt.float16
f8 = mybir.dt.float8e4
np_f8 = ml_dtypes.float8_e4m3fn


# --------------------------------------------------------------- BIR fixup
# This toolchain's walrus encodes at most ONE sync wait per instruction
# ("Too many sync wait commands"), but Tile's add_semaphores freely
# attaches several. Hoist excess waits onto EventSemaphore carriers on
# the same engine, inserted just before the over-subscribed instruction.

def _split_excess_waits(bir_json: bytes) -> bytes:
    import json
    d = json.loads(bir_json)
    n_split = 0
    for fn in d.get("functions", []):
        for blk in fn.get("blocks", []):
            insts = blk.get("instructions")
            if not insts:
                continue
            out = []
            changed = False
            for ins in insts:
                si = ins.get("sync_info") or {}
                ow = si.get("on_wait") or []
                if len(ow) > 1 and "engine" in ins:
                    for w in ow[:-1]:
                        n_split += 1
                        out.append({
                            "debug": ins.get("debug", 0),
                            "engine": ins["engine"],
                            "ins": [],
                            "outs": [],
                            "name": f"{ins['name']}-xw{n_split}",
                            "opcode": "EventSemaphore",
                            "sync_info": {"on_update": [], "on_wait": [w]},
                        })
                    si["on_wait"] = [ow[-1]]
                    changed = True
                out.append(ins)
            if changed:
                blk["instructions"] = out
    if n_split == 0:
        return bir_json
    return json.dumps(d).encode()


_orig_compile_bir_kernel = bass_utils.compile_bir_kernel


def _patched_compile_bir_kernel(bir_json, tmpdir, neff_name="file.neff"):
    return _orig_compile_bir_kernel(_split_excess_waits(bir_json), tmpdir,
                                    neff_name)


def _install_birfix():
    from concourse import bass2jax
    if bass_utils.compile_bir_kernel.__name__ != "_patched_compile_bir_kernel":
        bass_utils.compile_bir_kernel = _patched_compile_bir_kernel
    if bass2jax.compile_bir_kernel.__name__ != "_patched_compile_bir_kernel":
        bass2jax.compile_bir_kernel = _patched_compile_bir_kernel


_install_birfix()


class FixedTileContext(TileContext):
    """Stock _drain_and_barrier stuffs every outstanding sem wait onto one
    SP Drain; TRN2 TPB_CTRL encoding only fits 1-2 sync waits and walrus
    dies with "Too many sync wait commands". Split the final global-clock
    wait into one Drain per logical proc."""

    def _drain_and_barrier(self, tick_clock, wait_clock):
        gc = tick_clock.global_clock
        n = len(gc)
        for p in range(n):
            if gc[p] > 0:
                vec = [0] * n
                vec[p] = gc[p]
                w = self.nc.sync.drain()
                wait_clock.add_sem_waits(w.ins, ScopedClock({None: VectorClock(vec)}))
        # The per-proc drains above run serially on SP, so every wait is
        # already satisfied here; emit the final drain bare.
        self.nc.sync.drain()
        self.nc.all_engine_barrier()
        assert self.sems is not None
        popped = self.nc._tile_sem_poison_stack.pop()
        assert popped is self._sem_poison
        self.nc.clear_and_free_semaphores(list(self.sems.allocated().values()))
        self.nc.all_engine_barrier()


# ---------------------------------------------------------------- host prep


def _plan_slots(grad_rows):
    """Degree-sort rows per batch into blocks, deal to cores, and compute
    the global per-slot degree D (equalized across all 8 cores)."""
    perms = []          # per batch: [TOTBLK*128] row ids (-1 = pad)
    degs = []
    d_blocks = np.zeros((B, 2, NBLK), np.int64)
    for b in range(B):
        deg = np.bincount(np.asarray(grad_rows[b]), minlength=N)
        order = np.argsort(-deg, kind="stable")
        perm = np.concatenate([order, np.full(TOTBLK * 128 - N, -1, np.int64)])
        dblk = deg[np.maximum(perm, 0)] * (perm >= 0)
        dblk = dblk.reshape(TOTBLK, 128).max(axis=1)
        for i in range(TOTBLK):
            d_blocks[b, i % 2, i // 2] = dblk[i]
        perms.append(perm)
        degs.append(deg)
    d_slots = np.maximum(d_blocks.max(axis=(0, 1)), 1)   # [NBLK]
    assert d_slots.max() <= 128, d_slots.max()
    return perms, degs, d_slots


def _slot_geometry(d_slots):
    """Per slot: D, rows-per-tile R, tiles T, stream tile offset, and the
    column offset of this slot's [selX_R | selY_R]-interleaved selector
    columns (2*T*R per slot)."""
    geo = []
    t_off = 0
    s_off = 0
    for D in d_slots.tolist():
        R = 128 // D
        T = math.ceil(128 / R)
        geo.append((D, R, T, t_off, s_off))
        t_off += T
        s_off += 2 * T * R
    return geo, t_off, s_off


def build_host_data(inputs):
    x_in = np.asarray(inputs["x_in"], np.float32)
    areas = np.asarray(inputs["vertex_areas"], np.float32)
    evals = np.asarray(inputs["evals"], np.float32)
    evecs = np.asarray(inputs["evecs"], np.float32)
    gxv = np.asarray(inputs["gradX_vals"], np.float32)
    gyv = np.asarray(inputs["gradY_vals"], np.float32)
    grows = np.asarray(inputs["grad_rows"], np.int64)
    gcols = np.asarray(inputs["grad_cols"], np.int64)
    times = np.clip(np.asarray(inputs["diffusion_times"], np.float32), 1e-8, None)
    W1 = np.asarray(inputs["W1"], np.float32)
    b1 = np.asarray(inputs["b1"], np.float32)
    W2 = np.asarray(inputs["W2"], np.float32)
    b2 = np.asarray(inputs["b2"], np.float32)
    W3 = np.asarray(inputs["W3"], np.float32)
    b3 = np.asarray(inputs["b3"], np.float32)
    B_re = np.asarray(inputs["B_re"], np.float32)
    B_im = np.asarray(inputs["B_im"], np.float32)

    perms, degs, d_slots = _plan_slots(grows)
    geo, TT, SELTOT = _slot_geometry(d_slots)

    # phase A inputs, partition-major: ax[p, c, 0, :] = evecs row c*128+p,
    # ax[p, c, 1, :] = (a*x_in) row c*128+p
    ax_all = np.zeros((B, NCHALF * 128, 2, P), np.float16)
    ax_all[:, :N, 0, :] = evecs.astype(np.float16)
    ax_all[:, :N, 1, :] = (x_in * areas[:, :, None]).astype(np.float16)
    ax_all = np.ascontiguousarray(
        ax_all.reshape(B, NCHALF, 128, 2, P).transpose(0, 2, 1, 3, 4))

    ident = np.eye(128, dtype=np.float16)

    in_maps = []
    core_perm = []
    for b in range(B):
        rows_b, cols_b = grows[b], gcols[b]
        esort = np.argsort(rows_b, kind="stable")
        deg = degs[b]
        rowptr = np.zeros(N + 1, np.int64)
        rowptr[1:] = np.cumsum(deg)
        scale = np.exp(-evals[b][:, None] * times[None, :]).astype(np.float32)
        ev8 = evecs[b].astype(np_f8)
        for h in range(2):
            blk_ids = 2 * np.arange(NBLK) + h          # block index within batch
            perm_own = perms[b].reshape(TOTBLK, 128)[blk_ids].reshape(-1)  # [ROWS]
            core_perm.append(perm_own)
            pv = np.maximum(perm_own, 0)
            valid = perm_own >= 0

            # per-row padded edge grid, slot by slot
            col_stream = np.zeros((TT, 128), np.int64)
            selxy = np.zeros((128, SELTOT), np.float32)
            for s, (D, R, T, toff, soff) in enumerate(geo):
                rows_blk = perm_own[s * 128:(s + 1) * 128]
                rb = np.maximum(rows_blk, 0)
                cnt = np.where(rows_blk >= 0, deg[rb], 0)
                assert cnt.max(initial=0) <= D
                idx = rowptr[rb][:, None] + np.arange(D)[None, :]
                mask = np.arange(D)[None, :] < cnt[:, None]
                eid = esort[np.where(mask, idx, 0)]
                cm = np.where(mask, cols_b[eid], 0)          # [128, D]
                vxm = np.where(mask, gxv[b][eid], 0.0)
                vym = np.where(mask, gyv[b][eid], 0.0)
                G = T * R
                pad = ((0, G - 128), (0, 0))
                cm = np.pad(cm, pad).reshape(T, R * D)
                col_stream[toff:toff + T] = np.pad(
                    cm, ((0, 0), (0, 128 - R * D)))
                # interleaved per-tile selectors [selX_R | selY_R]:
                # sel[e, i] = val[row jR+i, d] where (i, d) = divmod(e, D)
                vxm = np.pad(vxm, pad).reshape(T, R, D)      # [T, R, D]
                vym = np.pad(vym, pad).reshape(T, R, D)
                e = np.arange(128)
                ei, ed = e // D, e % D                       # row-in-tile, d
                emask = ei < R
                eis = np.where(emask, ei, 0)
                sx = vxm[:, eis, ed] * emask                 # [T, 128]
                sy = vym[:, eis, ed] * emask
                blkx = np.zeros((T, 128, R), np.float32)
                blky = np.zeros((T, 128, R), np.float32)
                blkx[:, e, eis] = sx * emask
                blky[:, e, eis] = sy * emask
                inter = np.concatenate([blkx, blky], axis=2)  # [T, 128, 2R]
                selxy[:, soff:soff + 2 * T * R] = (
                    inter.transpose(1, 0, 2).reshape(128, T * 2 * R))

            evg = np.ascontiguousarray(
                ev8[col_stream].transpose(1, 0, 2))          # [128, TT, K] fp8

            in_maps.append({
                "evg": evg,
                "selxy": selxy.astype(np_f8),
                "ax": ax_all[b],
                "evsT": np.ascontiguousarray(
                    (evecs[b][pv].T * valid[None, :]).astype(np.float16)),
                "xinT": np.ascontiguousarray(
                    (x_in[b][pv].T * valid[None, :]).astype(np.float16)),
                "scale": scale,
                "ident": ident,
                "w1t": np.ascontiguousarray(W1.T.reshape(3, P, P).astype(np.float16)),
                "w2t": np.ascontiguousarray(W2.T.astype(np.float16)),
                "w3t": np.ascontiguousarray(W3.T.astype(np.float16)),
                "bret": np.ascontiguousarray(B_re.T.astype(np.float16)),
                "bimt": np.ascontiguousarray(B_im.T.astype(np.float16)),
                "b1": b1.reshape(P, 1).copy(),
                "b2": b2.reshape(P, 1).copy(),
                "b3": b3.reshape(P, 1).copy(),
            })

    meta = {"geo": geo, "TT": TT, "SELTOT": SELTOT, "d_slots": d_slots}
    return in_maps, core_perm, meta


# ------------------------------------------------------------ device kernel


def build_bass(meta):
    geo = meta["geo"]
    TT = meta["TT"]
    SELTOT = meta["SELTOT"]

    nc = bass.Bass("TRN2", target_bir_lowering=False, debug=False,
                   num_devices=NCORES)

    evg_d = nc.dram_tensor("evg", [128, TT, K], f8, kind="ExternalInput")
    selxy_d = nc.dram_tensor("selxy", [128, SELTOT], f8, kind="ExternalInput")
    ax_d = nc.dram_tensor("ax", [128, NCHALF, 2, P], f16, kind="ExternalInput")
    evsT_d = nc.dram_tensor("evsT", [K, ROWS], f16, kind="ExternalInput")
    xinT_d = nc.dram_tensor("xinT", [P, ROWS], f16, kind="ExternalInput")
    scale_d = nc.dram_tensor("scale", [K, P], f32, kind="ExternalInput")
    ident_d = nc.dram_tensor("ident", [128, 128], f16, kind="ExternalInput")
    w1t_d = nc.dram_tensor("w1t", [3, P, P], f16, kind="ExternalInput")
    w2t_d = nc.dram_tensor("w2t", [P, P], f16, kind="ExternalInput")
    w3t_d = nc.dram_tensor("w3t", [P, P], f16, kind="ExternalInput")
    bret_d = nc.dram_tensor("bret", [P, P], f16, kind="ExternalInput")
    bimt_d = nc.dram_tensor("bimt", [P, P], f16, kind="ExternalInput")
    b1_d = nc.dram_tensor("b1", [P, 1], f32, kind="ExternalInput")
    b2_d = nc.dram_tensor("b2", [P, 1], f32, kind="ExternalInput")
    b3_d = nc.dram_tensor("b3", [P, 1], f32, kind="ExternalInput")
    outT_d = nc.dram_tensor("outT", [P, ROWS], f16, kind="ExternalOutput")

    AF = mybir.ActivationFunctionType
    SCR = 64       # de-interleave overrun scratch columns
    ACH = 16       # phase A chunks per DMA (8KB/partition descriptors)
    NOCT = (NCHALF + ACH - 1) // ACH
    XCH = 64       # evg tiles per DMA (fp8: 8KB/partition descriptors)
    NXB = (TT + XCH - 1) // XCH

    with FixedTileContext(nc) as tc:
        with (
            tc.tile_pool(name="consts", bufs=1) as cpool,
            tc.tile_pool(name="resid", bufs=1) as rpool,
        ):
            # ---- constants
            scale_t = cpool.tile([K, P], f32, tag="scale")
            nc.sync.dma_start(scale_t[:], scale_d[:])
            ident_t = cpool.tile([128, 128], f16, tag="ident")
            nc.sync.dma_start(ident_t[:], ident_d[:])
            wh = cpool.tile([P, 5, P], f16, tag="wh")
            nc.sync.dma_start(wh[:, 0:3, :], w1t_d[:].rearrange("s p q -> p s q"))
            nc.sync.dma_start(wh[:, 3, :], w2t_d[:])
            nc.sync.dma_start(wh[:, 4, :], w3t_d[:])
            w1a_t = wh[:, 0, :]
            w1b_t = wh[:, 1, :]
            w1c_t = wh[:, 2, :]
            w2t_t = wh[:, 3, :]
            w3t_t = wh[:, 4, :]
            bh = cpool.tile([P, 2, P], f16, tag="bh")
            nc.sync.dma_start(bh[:, 0, :], bret_d[:])
            nc.sync.dma_start(bh[:, 1, :], bimt_d[:])
            bret_t = bh[:, 0, :]
            bimt_t = bh[:, 1, :]
            b1_t = cpool.tile([P, 1], f32, tag="b1")
            nc.sync.dma_start(b1_t[:], b1_d[:])
            b2_t = cpool.tile([P, 1], f32, tag="b2")
            nc.sync.dma_start(b2_t[:], b2_d[:])
            b3_t = cpool.tile([P, 1], f32, tag="b3")
            nc.sync.dma_start(b3_t[:], b3_d[:])

            # ---- resident SBUF state
            agX_t = rpool.tile([K, ROWS + SCR], f16, tag="agX")
            agY_t = rpool.tile([K, ROWS + SCR], f16, tag="agY")
            s2acc = rpool.tile([K, P], f32, tag="s2acc")
            s2h_t = rpool.tile([K, P], f16, tag="s2h")

            folds = rpool.tile([K, 3, P], f16, tag="folds")
            bfx_t = folds[:, 0, :]
            bfy_t = folds[:, 1, :]
            wf_t = folds[:, 2, :]

            # ---- phase 1 (C1 segment sums, phase A) merged with phase 2:
            # after the folds are ready (~group MERGE0), each loop iteration
            # emits one phase-2 group (10 groups behind) before the next
            # phase-1 group, so phase-2 compute fills phase-1's DMA waits.
            MERGE0 = 5     # phase-1-only groups; 2 A-octets processed per group
            with (
                tc.tile_pool(name="pX", bufs=5) as pX,
                tc.tile_pool(name="pS", bufs=3) as pS,
                tc.tile_pool(name="psAG", bufs=2, space="PSUM") as psAG_pool,
            ):
                # evg stream, XCH tiles per DMA, 2 blocks of lookahead
                evg_tiles = {}
                evg_next = [0]

                def evg_fetch():
                    b = evg_next[0]
                    if b >= NXB:
                        return
                    t0 = b * XCH
                    w = min(XCH, TT - t0)
                    xt = pX.tile([128, XCH, K], f8, tag="evg")
                    eng = nc.sync if b % 2 == 0 else nc.scalar
                    eng.dma_start(xt[:, :w], evg_d[:, t0:t0 + w])
                    evg_tiles[t0] = xt
                    evg_next[0] = b + 1

                def evg_tile(t):
                    t0 = (t // XCH) * XCH
                    while t0 not in evg_tiles:
                        evg_fetch()
                    while evg_next[0] < NXB and evg_next[0] * XCH <= t0 + 3 * XCH:
                        evg_fetch()
                    for k in list(evg_tiles):
                        if k < t0:
                            del evg_tiles[k]
                    return evg_tiles[t0][:, t - t0, :]

                # selector stream, prefetched two groups ahead
                selg_tiles = {}
                mxsel = max(2 * T * R for (D, R, T, _, _) in geo)

                def selg_fetch(gi):
                    g = gi * GRP
                    if g >= NBLK or gi in selg_tiles:
                        return
                    nb = min(GRP, NBLK - g)
                    sel0 = geo[g][4]
                    sel1 = (geo[g + nb][4] if g + nb < NBLK else SELTOT)
                    st = pS.tile([128, GRP * mxsel], f8, tag="selg")
                    nc.scalar.dma_start(st[:, :sel1 - sel0],
                                        selxy_d[:, sel0:sel1])
                    selg_tiles[gi] = (st, sel0)

                def p1_group(gi):
                    g = gi * GRP
                    nb = min(GRP, NBLK - g)
                    g0 = g * 128
                    selg_fetch(gi + 1)
                    selg_fetch(gi + 2)
                    selg, sel0 = selg_tiles.pop(gi)
                    for q in range(nb):
                        s = g + q
                        D, R, T, toff, soff = geo[s]
                        so = soff - sel0
                        agXY = psAG_pool.tile([K, 2 * T * R], f32, tag="agXY")
                        for j in range(T):
                            nc.tensor.matmul(
                                agXY[:, 2 * j * R:2 * (j + 1) * R],
                                evg_tile(toff + j),
                                selg[:, so + 2 * j * R:so + 2 * (j + 1) * R],
                                start=True, stop=True,
                            )
                        # de-interleave [X_R | Y_R]*T -> row-contiguous halves
                        agv = agXY[:].rearrange("k (t x) -> k t x", x=2 * R)
                        c0 = g0 + q * 128
                        nc.vector.tensor_copy(agX_t[:, c0:c0 + T * R],
                                              agv[:, :, 0:R])
                        nc.scalar.copy(agY_t[:, c0:c0 + T * R],
                                       agv[:, :, R:2 * R])

                with (
                    tc.tile_pool(name="pA", bufs=5) as pA,
                    tc.tile_pool(name="pF", bufs=1) as pF,
                    tc.tile_pool(name="psA", bufs=2, space="PSUM") as psA_pool,
                    tc.tile_pool(name="psF", bufs=2, space="PSUM") as psF_pool,
                ):
                    ax_q = []

                    def a_dma(k):
                        if k >= NOCT:
                            return
                        w = min(ACH, NCHALF - k * ACH)
                        t = pA.tile([128, ACH, 2, P], f16, tag="ax")
                        eng = nc.sync if k % 2 else nc.gpsimd
                        eng.dma_start(t[:, :w],
                                      ax_d[:, k * ACH:k * ACH + w])
                        ax_q.append((t, w, k))

                    def a_mms():
                        t, w, k = ax_q.pop(0)
                        ps = psA_pool.tile([K, P], f32, tag="psA")
                        for i in range(w):
                            nc.tensor.matmul(ps[:], t[:, i, 0, :],
                                             t[:, i, 1, :],
                                             start=(i == 0), stop=(i == w - 1))
                        if k == 0:
                            nc.vector.tensor_copy(s2acc[:], ps[:])
                        else:
                            nc.vector.tensor_add(s2acc[:], s2acc[:], ps[:])

                    a_dma(0)
                    a_dma(1)
                    a_dma(2)
                    a_dma(3)
                    evg_fetch()
                    evg_fetch()
                    selg_fetch(0)
                    for gi in range(MERGE0):
                        if ax_q:
                            a_mms()
                        if ax_q:
                            a_mms()
                        p1_group(gi)
                        a_dma(2 * gi + 4)
                        a_dma(2 * gi + 5)
                    while ax_q:
                        a_mms()

                    # folds: s2, s2^T, bfx/bfy/wf
                    s2T_t = pF.tile([P, K], f16, tag="s2T")
                    nc.vector.tensor_mul(s2acc[:], scale_t[:], s2acc[:])
                    nc.vector.tensor_copy(s2h_t[:], s2acc[:])
                    psT = psF_pool.tile([P, K], f16, tag="psT")
                    nc.tensor.transpose(psT[:], s2h_t[:], ident_t[:])
                    nc.scalar.copy(s2T_t[:], psT[:])
                    psF1 = psF_pool.tile([K, P], f32, tag="psF")
                    nc.tensor.matmul(psF1[:], s2T_t[:], bret_t[:],
                                     start=True, stop=True)
                    nc.scalar.copy(bfx_t[:], psF1[:])
                    psF2 = psF_pool.tile([K, P], f32, tag="psF")
                    nc.tensor.matmul(psF2[:], s2T_t[:], bimt_t[:],
                                     start=True, stop=True)
                    nc.scalar.copy(bfy_t[:], psF2[:])
                    psF3 = psF_pool.tile([K, P], f32, tag="psF")
                    nc.tensor.matmul(psF3[:], s2T_t[:], w1b_t[:],
                                     start=True, stop=True)
                    nc.scalar.copy(wf_t[:], psF3[:])

                # ---- phase 2 groups, interleaved with remaining phase 1
                with (
                    tc.tile_pool(name="pG", bufs=3) as pG,
                    tc.tile_pool(name="pE", bufs=3) as pE,
                    tc.tile_pool(name="psGX", bufs=1, space="PSUM") as psGX_pool,
                    tc.tile_pool(name="psGY", bufs=1, space="PSUM") as psGY_pool,
                    tc.tile_pool(name="psBX", bufs=1, space="PSUM") as psBX_pool,
                    tc.tile_pool(name="psBY", bufs=1, space="PSUM") as psBY_pool,
                    tc.tile_pool(name="psH", bufs=2, space="PSUM") as psH_pool,
                ):
                    ei_tiles = {}

                    def ei_fetch(gi):
                        g = gi * GRP
                        if g >= NBLK or gi in ei_tiles:
                            return
                        nb = min(GRP, NBLK - g)
                        gw = nb * 128
                        g0 = g * 128
                        et = pE.tile([K, GRP * 128], f16, tag="evsTg")
                        xt = pE.tile([P, GRP * 128], f16, tag="xinTg")
                        nc.gpsimd.dma_start(et[:, :gw], evsT_d[:, g0:g0 + gw])
                        nc.gpsimd.dma_start(xt[:, :gw], xinT_d[:, g0:g0 + gw])
                        ei_tiles[gi] = (et, xt)

                    def p2_group(gi):
                        g = gi * GRP
                        nb = min(GRP, NBLK - g)
                        gw = nb * 128
                        g0 = g * 128
                        ei_fetch(gi + 2)
                        evsT_g, xinT_g = ei_tiles.pop(gi)
                        agXs = agX_t[:, g0:g0 + gw]
                        agYs = agY_t[:, g0:g0 + gw]
                        psGX = psGX_pool.tile([P, GRP * 128], f32, tag="psGX")
                        psGY = psGY_pool.tile([P, GRP * 128], f32, tag="psGY")
                        psBX = psBX_pool.tile([P, GRP * 128], f32, tag="psBX")
                        psBY = psBY_pool.tile([P, GRP * 128], f32, tag="psBY")
                        nc.tensor.matmul(psGX[:, :gw], s2h_t[:], agXs,
                                         start=True, stop=True)
                        nc.tensor.matmul(psGY[:, :gw], s2h_t[:], agYs,
                                         start=True, stop=True)
                        nc.tensor.matmul(psBX[:, :gw], bfx_t[:], agXs,
                                         start=True, stop=True)
                        nc.tensor.matmul(psBY[:, :gw], bfy_t[:], agYs,
                                         start=True, stop=True)
                        gxh = pG.tile([P, GRP * 128], f16, tag="gxh")
                        gyh = pG.tile([P, GRP * 128], f16, tag="gyh")
                        t1 = pG.tile([P, GRP * 128], f32, tag="t1")
                        t2 = pG.tile([P, GRP * 128], f32, tag="t2")
                        # X-side first so psGX/psBX free before the Y ops
                        nc.vector.tensor_copy(gxh[:, :gw], psGX[:, :gw])
                        nc.vector.tensor_mul(t1[:, :gw], gxh[:, :gw],
                                             psBX[:, :gw])
                        nc.vector.tensor_copy(gyh[:, :gw], psGY[:, :gw])
                        nc.vector.tensor_mul(t2[:, :gw], gyh[:, :gw],
                                             psBY[:, :gw])
                        nc.vector.tensor_add(t1[:, :gw], t1[:, :gw],
                                             t2[:, :gw])
                        xg_sb = pG.tile([P, GRP * 128], f16, tag="xg")
                        nc.scalar.activation(xg_sb[:, :gw], t1[:, :gw],
                                             AF.Tanh)

                        # MLP + residual
                        psH1 = psH_pool.tile([P, GRP * 128], f32, tag="psH")
                        nc.tensor.matmul(psH1[:, :gw], w1a_t,
                                         xinT_g[:, :gw], start=True,
                                         stop=False)
                        nc.tensor.matmul(psH1[:, :gw], wf_t,
                                         evsT_g[:, :gw],
                                         start=False, stop=False)
                        nc.tensor.matmul(psH1[:, :gw], w1c_t,
                                         xg_sb[:, :gw], start=False, stop=True)
                        h_sb = pG.tile([P, GRP * 128], f16, tag="h")
                        nc.scalar.activation(h_sb[:, :gw], psH1[:, :gw],
                                             AF.Relu, bias=b1_t[:])
                        psH2 = psH_pool.tile([P, GRP * 128], f32, tag="psH")
                        nc.tensor.matmul(psH2[:, :gw], w2t_t,
                                         h_sb[:, :gw], start=True, stop=True)
                        h2_sb = pG.tile([P, GRP * 128], f16, tag="h")
                        nc.scalar.activation(h2_sb[:, :gw], psH2[:, :gw],
                                             AF.Relu, bias=b2_t[:])
                        psH3 = psH_pool.tile([P, GRP * 128], f32, tag="psH")
                        nc.tensor.matmul(psH3[:, :gw], w3t_t,
                                         h2_sb[:, :gw], start=True, stop=False)
                        nc.tensor.matmul(psH3[:, :gw], ident_t[:],
                                         xinT_g[:, :gw], start=False,
                                         stop=True)
                        out_sb = pG.tile([P, GRP * 128], f16, tag="out")
                        nc.scalar.activation(out_sb[:, :gw], psH3[:, :gw],
                                             AF.Identity, bias=b3_t[:])
                        nc.gpsimd.dma_start(outT_d[:, g0:g0 + gw],
                                            out_sb[:, :gw])

                    ei_fetch(0)
                    ei_fetch(1)
                    for gi in range(MERGE0, NGRP):
                        p2_group(gi - MERGE0)
                        p1_group(gi)
                    for gi in range(NGRP - MERGE0, NGRP):
                        p2_group(gi)

    return nc


# ---------------------------------------------------------------- top level

_CACHE = {}


def _get_bass(meta):
    key = tuple(meta["d_slots"].tolist())
    if key not in _CACHE:
        _CACHE[key] = build_bass(meta)
    return _CACHE[key]


def kernel(_trace=False, **inputs):
    in_maps, core_perm, meta = build_host_data(inputs)
    nc = _get_bass(meta)
    res = bass_utils.run_bass_kernel_spmd(
        nc, in_maps, core_ids=list(range(NCORES)), trace=_trace,
        trace_cores=list(range(NCORES)) if _trace else None,
    )
    out = np.zeros((B, N, P), np.float32)
    for c in range(NCORES):
        b = c // 2
        perm = core_perm[c]
        valid = perm >= 0
        outT = res.results[c]["outT"]           # [P, ROWS] f16
        out[b, perm[valid]] = outT.T[valid].astype(np.float32)
    if _trace:
        return out, res
    return out
